# revision 32
# baseline (speedup 1.0000x reference)
"""BERT self-attention (B=4, S=2048, HID=768, 12 heads) on 8 NeuronCores.

Sharding: data-parallel over batch (4) x tensor-parallel over heads (2 groups
of 6 heads) -> 8 cores, no cross-core communication.

Design (v3): row-tiled scores + cross-block ctx pipeline.
- All PE operands bf16; accumulation fp32 in PSUM.  Host pre-casts hs/W to
  bf16 and pre-transposes hs.
- Scores matmuls run as ROW-TILED PAIRS: the two heads of a pair occupy PE
  row-tiles (0,0)/(64,0) (K=64 each) and execute concurrently -> scores cost
  halves vs the zero-padded 128-contraction layout.  K^T for a head pair
  lives stacked in one [128, S] tile (rows 0:64 head even, 64:128 head odd),
  written by a single projection copy.
- exp split between Activation (native Exp, mask as bias) and DVE
  (Schraudolph bf16-bits trick), one engine per head each kc slot.
- ctx of block i (stationary = flat [v|1] V windows, probs moving) is
  emitted inside block i+1's scores loop: each kc slot = 4 row-tiled score
  MMs + exps + a 128-mode pump (4 ctx MMs h0 first half / h1 second half
  through 2 rotating PSUM bufs, plus projection/V fillers and tails).  The
  probs live in a 3-deep ring of [128, 8, 1024] half-tiles per head lane.
- Tails: PE-transpose ctx^T back to seq-major, DVE reciprocal of the
  denominator column, per-slice normalize into bf16 out tiles (host upcasts).
"""

import numpy as np

import concourse.bacc as bacc
import concourse.mybir as mybir
import concourse.tile as tile
from concourse.bass_utils import run_bass_kernel_spmd
from concourse.masks import make_identity

F32 = mybir.dt.float32
BF16 = mybir.dt.bfloat16
U16 = mybir.dt.uint16
EXP = mybir.ActivationFunctionType.Exp
MULT = mybir.AluOpType.mult
ADD = mybir.AluOpType.add

B = 4
S = 2048
HID = 768
NH_FULL = 12
HD = 64
NCORES = 8
NH = 6              # heads per core
D3 = NH * HD        # 384, per-core projection width
ST = S // 128       # 16 seq tiles
QB = 1024           # query block (2 x 512 matmul chunks)
QC = 512            # max moving-operand width per fp32-psum matmul
NQB = S // QB       # 2
KC = S // 128       # 16 key chunks
VW = NH * (HD + 1) + 63  # 453: flat [v|1]x6 + zero tail

# Schraudolph exp in bf16-bits domain: bf16_bits(exp(s)) ~ s*A + B0
SCHR_A = float(128.0 / np.log(2.0))       # 184.6650...
SCHR_B = float(127 * 128 - 7.366)         # offset calibrated for ZERO-MEAN rel err
# kc slot -> engine for head-0's exp ('a' = Activation, 'v' = DVE); head-1
# gets the other engine.
EXP_ENG = "avavavavavavavav"

_nc_cache: dict = {}


def _build(ck: int):
    """Build the per-core program. ck = # of 128-row contraction chunks in the
    projection (6 plain, 7 when biases are folded in via an augmented row)."""
    nc = bacc.Bacc("TRN2", target_bir_lowering=False, debug=False)
    hst_d = nc.dram_tensor("hst", [ck * 128, S], BF16, kind="ExternalInput")
    wq_d = nc.dram_tensor("wq", [ck * 128, D3], BF16, kind="ExternalInput")
    wk_d = nc.dram_tensor("wk", [ck * 128, D3], BF16, kind="ExternalInput")
    wv_d = nc.dram_tensor("wv", [ck * 128, D3], BF16, kind="ExternalInput")
    mask_d = nc.dram_tensor("mask", [128, KC], F32, kind="ExternalInput")
    msch_d = nc.dram_tensor("msch", [128, KC], F32, kind="ExternalInput")
    out_d = nc.dram_tensor("out", [S, D3], BF16, kind="ExternalOutput")

    with tile.TileContext(nc) as tc:
        with (
            tc.tile_pool(name="const", bufs=1) as constp,
            tc.tile_pool(name="qkpool", bufs=1) as qkp,
            tc.tile_pool(name="vpool", bufs=1) as vp,
            tc.tile_pool(name="hstpool", bufs=1) as hstp,
            tc.tile_pool(name="wpool", bufs=1) as wp,
            tc.tile_pool(name="outpool", bufs=1) as outp,
            tc.tile_pool(name="prpool", bufs=3) as prp,
            tc.tile_pool(name="ctxtpool", bufs=3) as ctxtp,
            tc.tile_pool(name="rdpool", bufs=4) as rdp,
            tc.tile_pool(name="stps", bufs=1, space="PSUM") as stps,
            tc.tile_pool(name="wps", bufs=2, space="PSUM") as wps,
            tc.tile_pool(name="cpsp", bufs=2, space="PSUM") as cpsp,
        ):
            identity = constp.tile([128, 128], BF16)
            make_identity(nc, identity)
            mask_sb = constp.tile([128, KC], F32)
            msch_sb = constp.tile([128, KC], F32)

            hsT = [hstp.tile([128, S], BF16, name=f"hsT{c}") for c in range(ck)]
            wq_sb = wp.tile([128, ck, D3], BF16, name="wq_sb")
            wk_sb = wp.tile([128, ck, D3], BF16, name="wk_sb")
            wv_sb = wp.tile([128, ck, D3], BF16, name="wv_sb")

            qt = [qkp.tile([128, S], BF16, name=f"qt{m}") for m in range(3)]
            # head-pair K^T stationaries: rows 0:64 = head 2m, 64:128 = 2m+1
            kt2 = [qkp.tile([128, S], BF16, name=f"kt2_{m}") for m in range(3)]

            v_sb = [vp.tile([128, VW], BF16, name=f"v{i}") for i in range(ST)]

            ob = [outp.tile([128, ST // 2, D3], BF16, name=f"ob{i}") for i in range(2)]
            out_sb = [ob[i // (ST // 2)][:, i % (ST // 2), :] for i in range(ST)]

            # ---- input DMAs over the 3 DMA-capable queues (sync/scalar/
            # gpsimd).  wq arrives in per-pair column slices so the pair-0
            # projection starts ~3us in; hsT arrives in 512-col pieces.
            # gpsimd first issues wv, then runs the v_sb memsets. ----
            wq_r = wq_d.ap().rearrange("(c p) (m n) -> p c m n", p=128, m=3)
            wq_sb4 = wq_sb.rearrange("p c (m n) -> p c m n", m=3)
            nc.scalar.dma_start(wq_sb4[:, :, 0, :], wq_r[:, :, 0, :])
            nc.sync.dma_start(mask_sb[:], mask_d[:])
            nc.sync.dma_start(msch_sb[:], msch_d[:])
            hst_r = hst_d.ap().rearrange("(c p) s -> p c s", p=128)
            for c in range(ck):
                eng = nc.sync if c % 2 == 0 else nc.scalar
                eng.dma_start(hsT[c][:, 0:QC], hst_r[:, c, 0:QC])
            nc.scalar.dma_start(wq_sb4[:, :, 1, :], wq_r[:, :, 1, :])
            nc.scalar.dma_start(wq_sb4[:, :, 2, :], wq_r[:, :, 2, :])
            nc.sync.dma_start(
                wk_sb[:], wk_d.ap().rearrange("(c p) n -> p c n", p=128))
            nc.gpsimd.dma_start(
                wv_sb[:], wv_d.ap().rearrange("(c p) n -> p c n", p=128))
            qs_ = (nc.sync, nc.scalar)
            for n in range(1, S // QC):
                for c in range(ck):
                    sl = slice(n * QC, (n + 1) * QC)
                    qs_[c % 2].dma_start(hsT[c][:, sl], hst_r[:, c, sl])
            for i in range(ST):
                v3 = v_sb[i][:, 0:NH * (HD + 1)].rearrange("p (h e) -> p h e", h=NH)
                nc.gpsimd.memset(v3[:, :, HD:HD + 1], 1.0)
                nc.gpsimd.memset(v_sb[i][:, NH * (HD + 1):VW].bitcast(U16), 0)

            # ---- emission helpers ----
            def emit_qk_pair_chunk(m, n):
                """Project q/k for head pair m, 512-query chunk n."""
                for which, w_sb in (("q", wq_sb), ("k", wk_sb)):
                    ps = wps.tile([128, QC], F32, name="ps", tag="mm")
                    for c in range(ck):
                        nc.tensor.matmul(
                            ps[:],
                            w_sb[:, c, m * 128:(m + 1) * 128],
                            hsT[c][:, n * QC:(n + 1) * QC],
                            start=(c == 0),
                            stop=(c == ck - 1),
                        )
                    sl = slice(n * QC, (n + 1) * QC)
                    dst = qt[m] if which == "q" else kt2[m]
                    nc.scalar.copy(dst[:, sl], ps[:])

            def emit_v_tile(st):
                vps = wps.tile([128, QC], F32, name="vps", tag="mm")
                for c in range(ck):
                    nc.tensor.matmul(
                        vps[:, 0:D3],
                        hsT[c][:, st * 128:(st + 1) * 128],
                        wv_sb[:, c, :],
                        start=(c == 0),
                        stop=(c == ck - 1),
                    )
                v3 = v_sb[st][:, 0:NH * (HD + 1)].rearrange("p (h e) -> p h e", h=NH)
                src = vps[:, 0:D3].rearrange("p (h d) -> p h d", h=NH)
                nc.scalar.copy(v3[:, :, 0:HD], src)

            E = HD + 2   # 66: keeps each tp2 slice 4-byte aligned in PSUM
            NQS = QB // 128

            def emit_tail(hp, hh, qb, ctxt):
                """transpose ctx^T back to seq-major, reciprocal of the
                denominator column, single fused broadcast-normalize."""
                h = 2 * hp + hh
                tp2 = wps.tile([128, NQS * E], BF16, name="tp2", tag="mm")
                for qs in range(NQS):
                    nc.tensor.transpose(
                        tp2[:, qs * E:qs * E + HD + 1],
                        ctxt[:, qs * 128:(qs + 1) * 128],
                        identity[0:HD + 1, 0:HD + 1],
                    )
                rd = rdp.tile([128, NQS], F32, name="rd")
                nc.vector.reciprocal(rd[:], tp2[:, HD::E])
                nc.vector.tensor_mul(
                    ob[qb][:, :, h * HD:(h + 1) * HD],
                    tp2.rearrange("p (q e) -> p q e", q=NQS)[:, :, 0:HD],
                    rd.rearrange("p (q o) -> p q o", o=1).to_broadcast(
                        [128, NQS, HD]),
                )

            out_r = out_d.ap().rearrange("(t p) n -> p t n", p=128)
            def emit_out_dma(qb):
                half = ST // 2
                for j in range(2):
                    lo = qb * half + j * (half // 2)
                    eng = nc.sync if j == 0 else nc.gpsimd
                    eng.dma_start(
                        out_r[:, lo:lo + half // 2, :],
                        ob[qb][:, j * (half // 2):(j + 1) * (half // 2), :])

            # ---- pre-loop: pair-0 projection chunks n0..n2 (DMA-paced);
            # n3 rides block 0's first pump slot ----
            for n in range(3):
                emit_qk_pair_chunk(0, n)

            # ---- main pipeline ----
            blocks = [(hp, qb) for hp in range(3) for qb in range(NQB)]
            prev = None         # (hp, qb, pr_halves) with ctx pending
            pending_tails = []  # deferred (hp, hh, qb, ctxt) tails

            def pump_fillers(bi, kc):
                """128-mode non-ctx PE work for slot kc of block bi."""
                if bi == 0:
                    if kc == 0:
                        emit_qk_pair_chunk(0, 3)
                    else:
                        emit_v_tile(kc - 1)          # st 0..14
                elif bi == 1:
                    if kc == 1:
                        emit_v_tile(15)
                    if kc % 2 == 0 and kc < 8:
                        emit_qk_pair_chunk(1, kc // 2)
                elif bi == 2:
                    if kc % 4 == 0:
                        emit_qk_pair_chunk(2, kc // 4)

            def emit_ctx_step(php, pqb, ppr, hh, k2, cps):
                """One ctx accumulation step (both 512-query chunks)."""
                h = 2 * php + hh
                for qc in range(QB // QC):
                    nc.tensor.matmul(
                        cps[qc][:],
                        v_sb[k2][:, h * (HD + 1):h * (HD + 1) + 128],
                        ppr[hh][k2 // 8][:, k2 % 8, qc * QC:(qc + 1) * QC],
                        start=(k2 == 0),
                        stop=(k2 == KC - 1),
                    )

            for bi, (hp, qb) in enumerate(blocks):
                pr_halves = [[None, None], [None, None]]  # [hh][half]
                cps = None
                for kc in range(KC):
                    # --- 128-mode pump FIRST: gives the previous slot's exps
                    # time to drain so the row-tiled pair below can actually
                    # overlap (its WAR waits are already satisfied) ---
                    pump_fillers(bi, kc)
                    if prev is not None:
                        php, pqb, ppr = prev
                        chh = kc // 8          # head lane: 0 first half, 1 second
                        for k2 in (2 * (kc % 8), 2 * (kc % 8) + 1):
                            if k2 == 0:
                                cps = [cpsp.tile([128, QC], F32, name="cps",
                                                 tag="cps")
                                       for _ in range(2)]
                            emit_ctx_step(php, pqb, ppr, chh, k2, cps)
                            if k2 == KC - 1:
                                ctxt = ctxtp.tile([HD + 1, QB], BF16, name="ctxt")
                                nc.vector.tensor_copy(
                                    ctxt[:, 0:QC], cps[0][0:HD + 1, :])
                                nc.vector.tensor_copy(
                                    ctxt[:, QC:QB], cps[1][0:HD + 1, :])
                                pending_tails.append((php, chh, pqb, ctxt))

                    # --- 64-mode: row-tiled score pair ---
                    sps0 = stps.tile([128, QB], F32, name="sps0", tag="s0")
                    sps1 = stps.tile([128, QB], F32, name="sps1", tag="s1")
                    if kc % 8 == 0:
                        pr_halves[0][kc // 8] = prp.tile(
                            [128, 8, QB], BF16, name="pr0", tag="pr0", bufs=3)
                        pr_halves[1][kc // 8] = prp.tile(
                            [128, 8, QB], BF16, name="pr1", tag="pr1", bufs=4)
                    ksl = slice(kc * 128, (kc + 1) * 128)
                    # palindrome order T0q0,T8q0,T8q1,T0q1: each tile's
                    # redundant LDWEIGHTS overlaps the OTHER tile's matmul
                    # (a same-row LDW would wait for its own in-flight MM)
                    def q_(qc):
                        return slice(qb * QB + qc * QC, qb * QB + (qc + 1) * QC)
                    def o_(qc):
                        return slice(qc * QC, (qc + 1) * QC)
                    nc.tensor.matmul(
                        sps0[:, o_(0)], kt2[hp][0:64, ksl], qt[hp][0:64, q_(0)],
                        start=True, stop=True,
                    )
                    nc.tensor.matmul(
                        sps1[:, o_(0)], kt2[hp][64:128, ksl], qt[hp][64:128, q_(0)],
                        start=True, stop=True,
                    )
                    nc.tensor.matmul(
                        sps1[:, o_(1)], kt2[hp][64:128, ksl], qt[hp][64:128, q_(1)],
                        start=True, stop=True,
                    )
                    nc.tensor.matmul(
                        sps0[:, o_(1)], kt2[hp][0:64, ksl], qt[hp][0:64, q_(1)],
                        start=True, stop=True,
                    )
                    # exp emitted as two 512-col half-ops: the next slot's
                    # q0 matmul only WAR-waits on the q0-half exp (slice-level
                    # dependency), releasing the scores->exp->scores chain
                    # ~0.6us earlier per slot.
                    for hh, sps in ((0, sps0), (1, sps1)):
                        if hh == 0:
                            eng = EXP_ENG[kc]
                        else:
                            eng = "v" if EXP_ENG[kc] == "a" else "a"
                        for half in range(2):
                            hsl = slice(half * QC, (half + 1) * QC)
                            prh = pr_halves[hh][kc // 8][:, kc % 8, hsl]
                            if eng == "a":
                                nc.scalar.activation(
                                    prh, sps[:, hsl], EXP,
                                    bias=mask_sb[:, kc:kc + 1], scale=1.0,
                                )
                            else:
                                nc.vector.tensor_scalar(
                                    prh.bitcast(U16), sps[:, hsl],
                                    SCHR_A, msch_sb[:, kc:kc + 1],
                                    op0=MULT, op1=ADD,
                                )

                    # tails: one deferred tail at slots 4 and 12
                    if pending_tails and kc in (4, 12):
                        emit_tail(*pending_tails.pop(0))

                prev = (hp, qb, pr_halves)

            # ---- post-loop: ctx + tails of the last block ----
            php, pqb, ppr = prev
            for hh in range(2):
                cps = [cpsp.tile([128, QC], F32, name="cps", tag="cps")
                       for _ in range(2)]
                for k2 in range(KC):
                    emit_ctx_step(php, pqb, ppr, hh, k2, cps)
                ctxt = ctxtp.tile([HD + 1, QB], BF16, name="ctxt")
                nc.scalar.copy(ctxt[:, 0:QC], cps[0][0:HD + 1, :])
                nc.vector.tensor_copy(ctxt[:, QC:QB], cps[1][0:HD + 1, :])
                if pending_tails:
                    emit_tail(*pending_tails.pop(0))   # block-4 h1 tail
                if hh == 0:
                    emit_out_dma(0)
                emit_tail(php, hh, pqb, ctxt)
            half = ST // 2
            for qs in range(QB // 128):
                sti = pqb * half + qs
                eng = nc.sync if qs % 2 == 0 else nc.gpsimd
                eng.dma_start(out_r[:, sti:sti + 1, :], ob[pqb][:, qs:qs + 1, :])

    nc.compile()
    return nc


def _get_nc(ck: int):
    if ck not in _nc_cache:
        _nc_cache[ck] = _build(ck)
    return _nc_cache[ck]


def _prepare_in_maps(hidden_states, attention_mask, Wq, bq, Wk, bk, Wv, bv):
    bf16 = mybir.dt.np(BF16)
    hs = np.asarray(hidden_states, dtype=np.float32)
    mask = np.asarray(attention_mask, dtype=np.float32).reshape(B, S)
    wq = np.asarray(Wq, dtype=np.float32) * np.float32(0.125)  # fold 1/sqrt(HD)
    wk = np.asarray(Wk, dtype=np.float32)
    wv = np.asarray(Wv, dtype=np.float32)
    bqs = np.asarray(bq, dtype=np.float32) * np.float32(0.125)
    bks = np.asarray(bk, dtype=np.float32)
    bvs = np.asarray(bv, dtype=np.float32)

    if bqs.any() or bks.any() or bvs.any():
        ck = 7
        pad = ck * 128 - (HID + 1)
        ones = np.ones((B, S, 1), np.float32)
        zer = np.zeros((B, S, pad), np.float32)
        hs = np.concatenate([hs, ones, zer], axis=2)
        def aug(w, b):
            return np.concatenate(
                [w, b[None, :], np.zeros((pad, HID), np.float32)], axis=0)
        wq, wk, wv = aug(wq, bqs), aug(wk, bks), aug(wv, bvs)
    else:
        ck = 6

    wq16 = wq.astype(bf16)
    wk16 = wk.astype(bf16)
    wv16 = wv.astype(bf16)
    msch = (np.float32(SCHR_B) + np.float32(SCHR_A) * mask).astype(np.float32)

    in_maps = []
    for core in range(NCORES):
        b, hg = core // 2, core % 2
        cols = slice(hg * D3, (hg + 1) * D3)
        in_maps.append({
            "hst": np.ascontiguousarray(hs[b].T.astype(bf16)),
            "wq": np.ascontiguousarray(wq16[:, cols]),
            "wk": np.ascontiguousarray(wk16[:, cols]),
            "wv": np.ascontiguousarray(wv16[:, cols]),
            "mask": np.ascontiguousarray(mask[b].reshape(KC, 128).T),
            "msch": np.ascontiguousarray(msch[b].reshape(KC, 128).T),
        })
    return ck, in_maps


def run(hidden_states, attention_mask, Wq, bq, Wk, bk, Wv, bv, **rb_kwargs):
    """Shard, run on 8 cores, gather. Returns (output, BassKernelResults)."""
    ck, in_maps = _prepare_in_maps(
        hidden_states, attention_mask, Wq, bq, Wk, bk, Wv, bv
    )
    nc = _get_nc(ck)
    res = run_bass_kernel_spmd(nc, in_maps, core_ids=list(range(NCORES)), **rb_kwargs)
    out = np.empty((B, S, HID), dtype=np.float32)
    for core in range(NCORES):
        b, hg = core // 2, core % 2
        out[b, :, hg * D3:(hg + 1) * D3] = res.results[core]["out"].astype(
            np.float32)
    return out, res


def kernel(hidden_states, attention_mask, Wq, bq, Wk, bk, Wv, bv):
    out, _ = run(hidden_states, attention_mask, Wq, bq, Wk, bk, Wv, bv)
    return out


# revision 34
# speedup vs baseline: 1.0714x; 1.0714x over previous
"""BERT self-attention (B=4, S=2048, HID=768, 12 heads) on 8 NeuronCores.

Sharding: data-parallel over batch (4) x tensor-parallel over heads (2 groups
of 6 heads) -> 8 cores, no cross-core communication.

Design (v3): row-tiled scores + cross-block ctx pipeline.
- All PE operands bf16; accumulation fp32 in PSUM.  Host pre-casts hs/W to
  bf16 and pre-transposes hs.
- Scores matmuls run as ROW-TILED PAIRS: the two heads of a pair occupy PE
  row-tiles (0,0)/(64,0) (K=64 each) and execute concurrently -> scores cost
  halves vs the zero-padded 128-contraction layout.  K^T for a head pair
  lives stacked in one [128, S] tile (rows 0:64 head even, 64:128 head odd),
  written by a single projection copy.
- exp split between Activation (native Exp, mask as bias) and DVE
  (Schraudolph bf16-bits trick), one engine per head each kc slot.
- ctx of block i (stationary = flat [v|1] V windows, probs moving) is
  emitted inside block i+1's scores loop: each kc slot = 4 row-tiled score
  MMs + exps + a 128-mode pump (4 ctx MMs h0 first half / h1 second half
  through 2 rotating PSUM bufs, plus projection/V fillers and tails).  The
  probs live in a 3-deep ring of [128, 8, 1024] half-tiles per head lane.
- Tails: PE-transpose ctx^T back to seq-major, DVE reciprocal of the
  denominator column, per-slice normalize into bf16 out tiles (host upcasts).
"""

import numpy as np

import concourse.bacc as bacc
import concourse.mybir as mybir
import concourse.tile as tile
from concourse.bass_utils import run_bass_kernel_spmd
from concourse.masks import make_identity

F32 = mybir.dt.float32
BF16 = mybir.dt.bfloat16
U16 = mybir.dt.uint16
EXP = mybir.ActivationFunctionType.Exp
MULT = mybir.AluOpType.mult
ADD = mybir.AluOpType.add

B = 4
S = 2048
HID = 768
NH_FULL = 12
HD = 64
NCORES = 8
NH = 6              # heads per core
D3 = NH * HD        # 384, per-core projection width
ST = S // 128       # 16 seq tiles
QB = 1024           # query block (2 x 512 matmul chunks)
QC = 512            # max moving-operand width per fp32-psum matmul
NQB = S // QB       # 2
KC = S // 128       # 16 key chunks
VW = NH * (HD + 1) + 63  # 453: flat [v|1]x6 + zero tail

# Schraudolph exp in bf16-bits domain: bf16_bits(exp(s)) ~ s*A + B0
SCHR_A = float(128.0 / np.log(2.0))       # 184.6650...
SCHR_B = float(127 * 128 - 7.366)         # offset calibrated for ZERO-MEAN rel err
# kc slot -> engine for head-0's exp ('a' = Activation, 'v' = DVE); head-1
# gets the other engine.
EXP_ENG = "avavavavavavavav"

_nc_cache: dict = {}


def _build(ck: int):
    """Build the per-core program. ck = # of 128-row contraction chunks in the
    projection (6 plain, 7 when biases are folded in via an augmented row)."""
    nc = bacc.Bacc("TRN2", target_bir_lowering=False, debug=False)
    hst_d = nc.dram_tensor("hst", [ck * 128, S], BF16, kind="ExternalInput")
    wq_d = nc.dram_tensor("wq", [ck * 128, D3], BF16, kind="ExternalInput")
    wk_d = nc.dram_tensor("wk", [ck * 128, D3], BF16, kind="ExternalInput")
    wv_d = nc.dram_tensor("wv", [ck * 128, D3], BF16, kind="ExternalInput")
    mask_d = nc.dram_tensor("mask", [128, KC], F32, kind="ExternalInput")
    msch_d = nc.dram_tensor("msch", [128, KC], F32, kind="ExternalInput")
    out_d = nc.dram_tensor("out", [S, D3], BF16, kind="ExternalOutput")

    with tile.TileContext(nc) as tc:
        with (
            tc.tile_pool(name="const", bufs=1) as constp,
            tc.tile_pool(name="qkpool", bufs=1) as qkp,
            tc.tile_pool(name="vpool", bufs=1) as vp,
            tc.tile_pool(name="hstpool", bufs=1) as hstp,
            tc.tile_pool(name="wpool", bufs=1) as wp,
            tc.tile_pool(name="outpool", bufs=1) as outp,
            tc.tile_pool(name="prpool", bufs=3) as prp,
            tc.tile_pool(name="ctxtpool", bufs=3) as ctxtp,
            tc.tile_pool(name="rdpool", bufs=4) as rdp,
            tc.tile_pool(name="stps", bufs=1, space="PSUM") as stps,
            tc.tile_pool(name="wps", bufs=2, space="PSUM") as wps,
            tc.tile_pool(name="cpsp", bufs=2, space="PSUM") as cpsp,
        ):
            identity = constp.tile([128, 128], BF16)
            make_identity(nc, identity)
            mask_sb = constp.tile([128, KC], F32)
            msch_sb = constp.tile([128, KC], F32)

            hsT = [hstp.tile([128, S], BF16, name=f"hsT{c}") for c in range(ck)]
            wq_sb = wp.tile([128, ck, D3], BF16, name="wq_sb")
            wk_sb = wp.tile([128, ck, D3], BF16, name="wk_sb")
            wv_sb = wp.tile([128, ck, D3], BF16, name="wv_sb")

            qt = [qkp.tile([128, S], BF16, name=f"qt{m}") for m in range(3)]
            # head-pair K^T stationaries: rows 0:64 = head 2m, 64:128 = 2m+1
            kt2 = [qkp.tile([128, S], BF16, name=f"kt2_{m}") for m in range(3)]

            v_sb = [vp.tile([128, VW], BF16, name=f"v{i}") for i in range(ST)]

            ob = [outp.tile([128, ST // 2, D3], BF16, name=f"ob{i}") for i in range(2)]
            out_sb = [ob[i // (ST // 2)][:, i % (ST // 2), :] for i in range(ST)]

            # ---- input DMAs over the 3 DMA-capable queues (sync/scalar/
            # gpsimd).  wq arrives in per-pair column slices so the pair-0
            # projection starts ~3us in; hsT arrives in 512-col pieces.
            # gpsimd first issues wv, then runs the v_sb memsets. ----
            wq_r = wq_d.ap().rearrange("(c p) (m n) -> p c m n", p=128, m=3)
            wq_sb4 = wq_sb.rearrange("p c (m n) -> p c m n", m=3)
            nc.scalar.dma_start(wq_sb4[:, :, 0, :], wq_r[:, :, 0, :])
            nc.sync.dma_start(mask_sb[:], mask_d[:])
            nc.sync.dma_start(msch_sb[:], msch_d[:])
            hst_r = hst_d.ap().rearrange("(c p) s -> p c s", p=128)
            for c in range(ck):
                eng = nc.sync if c % 2 == 0 else nc.scalar
                eng.dma_start(hsT[c][:, 0:QC], hst_r[:, c, 0:QC])
            nc.scalar.dma_start(wq_sb4[:, :, 1, :], wq_r[:, :, 1, :])
            nc.scalar.dma_start(wq_sb4[:, :, 2, :], wq_r[:, :, 2, :])
            nc.sync.dma_start(
                wk_sb[:], wk_d.ap().rearrange("(c p) n -> p c n", p=128))
            nc.gpsimd.dma_start(
                wv_sb[:], wv_d.ap().rearrange("(c p) n -> p c n", p=128))
            qs_ = (nc.sync, nc.scalar)
            for n in range(1, S // QC):
                for c in range(ck):
                    sl = slice(n * QC, (n + 1) * QC)
                    qs_[c % 2].dma_start(hsT[c][:, sl], hst_r[:, c, sl])
            for i in range(ST):
                v3 = v_sb[i][:, 0:NH * (HD + 1)].rearrange("p (h e) -> p h e", h=NH)
                nc.gpsimd.memset(v3[:, :, HD:HD + 1], 1.0)
                nc.gpsimd.memset(v_sb[i][:, NH * (HD + 1):VW].bitcast(U16), 0)

            # ---- emission helpers ----
            def emit_qk_pair_chunk(m, n):
                """Project q/k for head pair m, 512-query chunk n."""
                for which, w_sb in (("q", wq_sb), ("k", wk_sb)):
                    ps = wps.tile([128, QC], F32, name="ps", tag="mm")
                    for c in range(ck):
                        nc.tensor.matmul(
                            ps[:],
                            w_sb[:, c, m * 128:(m + 1) * 128],
                            hsT[c][:, n * QC:(n + 1) * QC],
                            start=(c == 0),
                            stop=(c == ck - 1),
                        )
                    sl = slice(n * QC, (n + 1) * QC)
                    dst = qt[m] if which == "q" else kt2[m]
                    nc.scalar.copy(dst[:, sl], ps[:])

            def emit_v_tile(st):
                vps = wps.tile([128, QC], F32, name="vps", tag="mm")
                for c in range(ck):
                    nc.tensor.matmul(
                        vps[:, 0:D3],
                        hsT[c][:, st * 128:(st + 1) * 128],
                        wv_sb[:, c, :],
                        start=(c == 0),
                        stop=(c == ck - 1),
                    )
                v3 = v_sb[st][:, 0:NH * (HD + 1)].rearrange("p (h e) -> p h e", h=NH)
                src = vps[:, 0:D3].rearrange("p (h d) -> p h d", h=NH)
                nc.scalar.copy(v3[:, :, 0:HD], src)

            E = HD + 2   # 66: keeps each tp2 slice 4-byte aligned in PSUM
            NQS = QB // 128

            def emit_tail(hp, hh, qb, ctxt):
                """transpose ctx^T back to seq-major, reciprocal of the
                denominator column, single fused broadcast-normalize."""
                h = 2 * hp + hh
                tp2 = wps.tile([128, NQS * E], BF16, name="tp2", tag="mm")
                for qs in range(NQS):
                    nc.tensor.transpose(
                        tp2[:, qs * E:qs * E + HD + 1],
                        ctxt[:, qs * 128:(qs + 1) * 128],
                        identity[0:HD + 1, 0:HD + 1],
                    )
                rd = rdp.tile([128, NQS], F32, name="rd")
                nc.vector.reciprocal(rd[:], tp2[:, HD::E])
                nc.vector.tensor_mul(
                    ob[qb][:, :, h * HD:(h + 1) * HD],
                    tp2.rearrange("p (q e) -> p q e", q=NQS)[:, :, 0:HD],
                    rd.rearrange("p (q o) -> p q o", o=1).to_broadcast(
                        [128, NQS, HD]),
                )

            out_r = out_d.ap().rearrange("(t p) n -> p t n", p=128)
            def emit_out_dma(qb):
                half = ST // 2
                for j in range(2):
                    lo = qb * half + j * (half // 2)
                    eng = nc.sync if j == 0 else nc.gpsimd
                    eng.dma_start(
                        out_r[:, lo:lo + half // 2, :],
                        ob[qb][:, j * (half // 2):(j + 1) * (half // 2), :])

            # ---- pre-loop: pair-0 projection chunks n0..n2 (DMA-paced);
            # n3 rides block 0's first pump slot ----
            for n in range(3):
                emit_qk_pair_chunk(0, n)

            # ---- main pipeline ----
            blocks = [(hp, qb) for hp in range(3) for qb in range(NQB)]
            prev = None         # (hp, qb, pr_halves) with ctx pending
            pending_tails = []  # deferred (hp, hh, qb, ctxt) tails

            def pump_fillers(bi, kc):
                """128-mode non-ctx PE work for slot kc of block bi."""
                if bi == 0:
                    if kc == 0:
                        emit_qk_pair_chunk(0, 3)
                    else:
                        emit_v_tile(kc - 1)          # st 0..14
                elif bi == 1:
                    if kc == 1:
                        emit_v_tile(15)
                    if kc % 2 == 0 and kc < 8:
                        emit_qk_pair_chunk(1, kc // 2)
                elif bi == 2:
                    if kc % 4 == 0:
                        emit_qk_pair_chunk(2, kc // 4)

            def emit_ctx_step(php, pqb, ppr, hh, k2, cps):
                """One ctx accumulation step (both 512-query chunks)."""
                h = 2 * php + hh
                for qc in range(QB // QC):
                    nc.tensor.matmul(
                        cps[qc][:],
                        v_sb[k2][:, h * (HD + 1):h * (HD + 1) + 128],
                        ppr[hh][k2 // 8][:, k2 % 8, qc * QC:(qc + 1) * QC],
                        start=(k2 == 0),
                        stop=(k2 == KC - 1),
                    )

            for bi, (hp, qb) in enumerate(blocks):
                pr_halves = [[None, None], [None, None]]  # [hh][half]
                cps = None
                for kc in range(KC):
                    # --- 128-mode pump FIRST: gives the previous slot's exps
                    # time to drain so the row-tiled pair below can actually
                    # overlap (its WAR waits are already satisfied) ---
                    pump_fillers(bi, kc)
                    if prev is not None:
                        php, pqb, ppr = prev
                        chh = kc // 8          # head lane: 0 first half, 1 second
                        for k2 in (2 * (kc % 8), 2 * (kc % 8) + 1):
                            if k2 == 0:
                                cps = [cpsp.tile([128, QC], F32, name="cps",
                                                 tag="cps")
                                       for _ in range(2)]
                            emit_ctx_step(php, pqb, ppr, chh, k2, cps)
                            if k2 == KC - 1:
                                ctxt = ctxtp.tile([HD + 1, QB], BF16, name="ctxt")
                                nc.scalar.copy(ctxt[:, 0:QC], cps[0][0:HD + 1, :])
                                nc.scalar.copy(ctxt[:, QC:QB], cps[1][0:HD + 1, :])
                                pending_tails.append((php, chh, pqb, ctxt))

                    # --- 64-mode: row-tiled score pair ---
                    sps0 = stps.tile([128, QB], F32, name="sps0", tag="s0")
                    sps1 = stps.tile([128, QB], F32, name="sps1", tag="s1")
                    if kc % 8 == 0:
                        pr_halves[0][kc // 8] = prp.tile(
                            [128, 8, QB], BF16, name="pr0", tag="pr0", bufs=3)
                        pr_halves[1][kc // 8] = prp.tile(
                            [128, 8, QB], BF16, name="pr1", tag="pr1", bufs=4)
                    ksl = slice(kc * 128, (kc + 1) * 128)
                    # palindrome order T0q0,T8q0,T8q1,T0q1: each tile's
                    # redundant LDWEIGHTS overlaps the OTHER tile's matmul
                    # (a same-row LDW would wait for its own in-flight MM)
                    def q_(qc):
                        return slice(qb * QB + qc * QC, qb * QB + (qc + 1) * QC)
                    def o_(qc):
                        return slice(qc * QC, (qc + 1) * QC)
                    nc.tensor.matmul(
                        sps0[:, o_(0)], kt2[hp][0:64, ksl], qt[hp][0:64, q_(0)],
                        start=True, stop=True,
                    )
                    nc.tensor.matmul(
                        sps1[:, o_(0)], kt2[hp][64:128, ksl], qt[hp][64:128, q_(0)],
                        start=True, stop=True,
                    )
                    nc.tensor.matmul(
                        sps1[:, o_(1)], kt2[hp][64:128, ksl], qt[hp][64:128, q_(1)],
                        start=True, stop=True,
                    )
                    nc.tensor.matmul(
                        sps0[:, o_(1)], kt2[hp][0:64, ksl], qt[hp][0:64, q_(1)],
                        start=True, stop=True,
                    )
                    # DVE exps emitted as two 512-col half-ops: the next
                    # slot's q0 matmul only WAR-waits on the q0-half exp
                    # (slice-level dependency), releasing the scores->exp->
                    # scores chain earlier.  ACT keeps one full-width op (its
                    # 352-cycle per-op overhead makes splitting a loss); the
                    # per-kc engine alternation halves the chain on average.
                    for hh, sps in ((0, sps0), (1, sps1)):
                        if hh == 0:
                            eng = EXP_ENG[kc]
                        else:
                            eng = "v" if EXP_ENG[kc] == "a" else "a"
                        if eng == "a":
                            nc.scalar.activation(
                                pr_halves[hh][kc // 8][:, kc % 8, :], sps[:],
                                EXP, bias=mask_sb[:, kc:kc + 1], scale=1.0,
                            )
                        else:
                            for half in range(2):
                                hsl = slice(half * QC, (half + 1) * QC)
                                prh = pr_halves[hh][kc // 8][:, kc % 8, hsl]
                                nc.vector.tensor_scalar(
                                    prh.bitcast(U16), sps[:, hsl],
                                    SCHR_A, msch_sb[:, kc:kc + 1],
                                    op0=MULT, op1=ADD,
                                )

                    # tails: one deferred tail at slots 4 and 12
                    if pending_tails and kc in (4, 12):
                        emit_tail(*pending_tails.pop(0))

                prev = (hp, qb, pr_halves)

            # ---- post-loop: ctx + tails of the last block ----
            php, pqb, ppr = prev
            for hh in range(2):
                cps = [cpsp.tile([128, QC], F32, name="cps", tag="cps")
                       for _ in range(2)]
                for k2 in range(KC):
                    emit_ctx_step(php, pqb, ppr, hh, k2, cps)
                ctxt = ctxtp.tile([HD + 1, QB], BF16, name="ctxt")
                nc.scalar.copy(ctxt[:, 0:QC], cps[0][0:HD + 1, :])
                nc.vector.tensor_copy(ctxt[:, QC:QB], cps[1][0:HD + 1, :])
                if pending_tails:
                    emit_tail(*pending_tails.pop(0))   # block-4 h1 tail
                if hh == 0:
                    emit_out_dma(0)
                emit_tail(php, hh, pqb, ctxt)
            half = ST // 2
            for qs in range(QB // 128):
                sti = pqb * half + qs
                eng = nc.sync if qs % 2 == 0 else nc.gpsimd
                eng.dma_start(out_r[:, sti:sti + 1, :], ob[pqb][:, qs:qs + 1, :])

    nc.compile()
    return nc


def _get_nc(ck: int):
    if ck not in _nc_cache:
        _nc_cache[ck] = _build(ck)
    return _nc_cache[ck]


def _prepare_in_maps(hidden_states, attention_mask, Wq, bq, Wk, bk, Wv, bv):
    bf16 = mybir.dt.np(BF16)
    hs = np.asarray(hidden_states, dtype=np.float32)
    mask = np.asarray(attention_mask, dtype=np.float32).reshape(B, S)
    wq = np.asarray(Wq, dtype=np.float32) * np.float32(0.125)  # fold 1/sqrt(HD)
    wk = np.asarray(Wk, dtype=np.float32)
    wv = np.asarray(Wv, dtype=np.float32)
    bqs = np.asarray(bq, dtype=np.float32) * np.float32(0.125)
    bks = np.asarray(bk, dtype=np.float32)
    bvs = np.asarray(bv, dtype=np.float32)

    if bqs.any() or bks.any() or bvs.any():
        ck = 7
        pad = ck * 128 - (HID + 1)
        ones = np.ones((B, S, 1), np.float32)
        zer = np.zeros((B, S, pad), np.float32)
        hs = np.concatenate([hs, ones, zer], axis=2)
        def aug(w, b):
            return np.concatenate(
                [w, b[None, :], np.zeros((pad, HID), np.float32)], axis=0)
        wq, wk, wv = aug(wq, bqs), aug(wk, bks), aug(wv, bvs)
    else:
        ck = 6

    wq16 = wq.astype(bf16)
    wk16 = wk.astype(bf16)
    wv16 = wv.astype(bf16)
    msch = (np.float32(SCHR_B) + np.float32(SCHR_A) * mask).astype(np.float32)

    in_maps = []
    for core in range(NCORES):
        b, hg = core // 2, core % 2
        cols = slice(hg * D3, (hg + 1) * D3)
        in_maps.append({
            "hst": np.ascontiguousarray(hs[b].T.astype(bf16)),
            "wq": np.ascontiguousarray(wq16[:, cols]),
            "wk": np.ascontiguousarray(wk16[:, cols]),
            "wv": np.ascontiguousarray(wv16[:, cols]),
            "mask": np.ascontiguousarray(mask[b].reshape(KC, 128).T),
            "msch": np.ascontiguousarray(msch[b].reshape(KC, 128).T),
        })
    return ck, in_maps


def run(hidden_states, attention_mask, Wq, bq, Wk, bk, Wv, bv, **rb_kwargs):
    """Shard, run on 8 cores, gather. Returns (output, BassKernelResults)."""
    ck, in_maps = _prepare_in_maps(
        hidden_states, attention_mask, Wq, bq, Wk, bk, Wv, bv
    )
    nc = _get_nc(ck)
    res = run_bass_kernel_spmd(nc, in_maps, core_ids=list(range(NCORES)), **rb_kwargs)
    out = np.empty((B, S, HID), dtype=np.float32)
    for core in range(NCORES):
        b, hg = core // 2, core % 2
        out[b, :, hg * D3:(hg + 1) * D3] = res.results[core]["out"].astype(
            np.float32)
    return out, res


def kernel(hidden_states, attention_mask, Wq, bq, Wk, bk, Wv, bv):
    out, _ = run(hidden_states, attention_mask, Wq, bq, Wk, bk, Wv, bv)
    return out
